# revision 11
# baseline (speedup 1.0000x reference)
"""Trainium2 Bass kernel for nn_CrossAttention (B=4, L=2048, D=1024, H=8).

Sharding: 8 cores = 4 batches x 2 query-halves (data parallel over B x Lq).
Each core computes, for its (b, half):
    Qn = LN(Q_slice); Kn = LN(K_b)            (pre_g folded into Wq/Wk rows,
                                               pre_b -> bias row on q/k)
    q = Qn @ Wq ; k = Kn @ Wk ; v = V_b @ Wv   (fp32r matmuls, feature-major)
    S_h^T = k_h q_h^T / TEMP  -> exp on ScalarE -> sums via ones-matmul on PE
    O^T_h = accumulation of v_h^T exp(S^T) on PE (unnormalized)
    O = LN(transpose(O^T) / sums) ; out = O + gelu(O @ Wo)

All heavy matmuls run in float32r (fp32 with 12-bit-truncated mantissa,
pre-rounded on host or rounded on-chip at PSUM evictions), which streams at
bf16 rate for free-dim >= 256.
"""

import numpy as np

P = 128
D = 1024
H = 8
HD = 128
LQ = 1024  # per-core query rows
LK = 2048
N_CORES = 8
TEMP = 32.0  # sqrt(D)
EPS = 1e-5

_PROGRAM_CACHE = {}


def round_fp32r(x: np.ndarray) -> np.ndarray:
    """Round fp32 to fp32r (12 low mantissa bits, round-to-nearest-even)."""
    u = np.ascontiguousarray(x).view(np.uint32)
    low = u & np.uint32(0xFFF)
    half = np.uint32(0x800)
    base = u & np.uint32(0xFFFFF000)
    rnd = np.where(
        (low > half)
        | ((low == half) & ((u >> np.uint32(12)) & np.uint32(1)).astype(bool)),
        base + np.uint32(0x1000),
        base,
    )
    return rnd.astype(np.uint32).view(np.float32)


def _build_program(has_qk_bias: bool, trivial_ln: bool):
    import concourse.bacc as bacc
    import concourse.mybir as mybir
    import concourse.tile as tile
    from contextlib import ExitStack

    FP32 = mybir.dt.float32
    FP32R = mybir.dt.float32r
    AF = mybir.ActivationFunctionType
    SUB = mybir.AluOpType.subtract
    MULT = mybir.AluOpType.mult

    nc = bacc.Bacc("TRN2", target_bir_lowering=False, debug=False)

    # ---- DRAM I/O ----
    Qs = nc.dram_tensor("Qs", [LQ, D], FP32, kind="ExternalInput")
    Kf = nc.dram_tensor("Kf", [LK, D], FP32, kind="ExternalInput")
    Vf = nc.dram_tensor("Vf", [LK, D], FP32, kind="ExternalInput")
    Wq_d = nc.dram_tensor("Wq_r", [D, D], FP32R, kind="ExternalInput")
    Wk_d = nc.dram_tensor("Wk_r", [D, D], FP32R, kind="ExternalInput")
    Wv_d = nc.dram_tensor("Wv_r", [D, D], FP32R, kind="ExternalInput")
    Wo_d = nc.dram_tensor("Wo_r", [D, D], FP32R, kind="ExternalInput")
    ID_R = nc.dram_tensor("ID_R", [P, P], FP32R, kind="ExternalInput")
    ID_F = nc.dram_tensor("ID_F", [P, P], FP32, kind="ExternalInput")
    ONES_D = nc.dram_tensor("ONES", [P, 1], FP32R, kind="ExternalInput")
    if has_qk_bias:
        BQ_D = nc.dram_tensor("BQ", [P, H], FP32, kind="ExternalInput")
        BK_D = nc.dram_tensor("BK", [P, H], FP32, kind="ExternalInput")
    if not trivial_ln:
        LNG_D = nc.dram_tensor("LNG_B", [P, D], FP32, kind="ExternalInput")
        LNB_D = nc.dram_tensor("LNB_B", [P, D], FP32, kind="ExternalInput")
    OUT = nc.dram_tensor("OUT", [LQ, D], FP32, kind="ExternalOutput")

    with tile.TileContext(nc) as tc, ExitStack() as top:
        singles = top.enter_context(tc.tile_pool(name="singles", bufs=1))
        dram_pool = top.enter_context(tc.tile_pool(name="dram", bufs=1, space="DRAM"))
        ident_r = singles.tile([P, P], FP32R)
        ident_f = singles.tile([P, P], FP32)
        ones = singles.tile([P, 1], FP32R)
        eps_t = singles.tile([P, 1], FP32)
        nc.sync.dma_start(ident_r[:], ID_R.ap())
        nc.sync.dma_start(ident_f[:], ID_F.ap())
        nc.sync.dma_start(ones[:], ONES_D.ap())
        nc.vector.memset(eps_t[:], EPS)
        if has_qk_bias:
            bq_sb = singles.tile([P, H], FP32)
            bk_sb = singles.tile([P, H], FP32)
            nc.sync.dma_start(bq_sb[:], BQ_D.ap())
            nc.sync.dma_start(bk_sb[:], BK_D.ap())
        sums_dram = dram_pool.tile([H, LQ], FP32R)
        ot_dram = dram_pool.tile([H, P, LQ], FP32R)

        def layernorm(pool, x_tile, out_tile):
            """LN over free dim (D=1024) of [128, D] fp32 tile -> fp32r out."""
            xr = x_tile[:].rearrange("p (n f) -> p n f", f=512)
            stats = pool.tile([P, 2, nc.vector.BN_STATS_DIM], FP32, tag="ln_stats")
            for i in range(2):
                nc.vector.bn_stats(out=stats[:, i, :], in_=xr[:, i, :])
            mv = pool.tile([P, nc.vector.BN_AGGR_DIM], FP32, tag="ln_mv")
            nc.vector.bn_aggr(out=mv[:], in_=stats[:])
            rstd = pool.tile([P, 1], FP32, tag="ln_rstd")
            nc.scalar.activation(
                out=rstd[:], in_=mv[:, 1:2], func=AF.Sqrt, bias=eps_t[:]
            )
            nc.vector.reciprocal(out=rstd[:], in_=rstd[:])
            nc.vector.tensor_scalar(
                out=out_tile[:],
                in0=x_tile[:],
                scalar1=mv[:, 0:1],
                scalar2=rstd[:],
                op0=SUB,
                op1=MULT,
            )

        act_store = ExitStack()
        kth_pool = act_store.enter_context(tc.tile_pool(name="kth", bufs=8))
        qth_pool = act_store.enter_context(tc.tile_pool(name="qth", bufs=8))
        kth = [
            kth_pool.tile([P, LK], FP32R, tag="kth", name=f"kth{h}")
            for h in range(H)
        ]
        qth = [
            qth_pool.tile([P, LQ], FP32R, tag="qth", name=f"qth{h}")
            for h in range(H)
        ]

        # ============ P1/P2: LN + transpose + q/k projections ============
        with ExitStack() as kq_ctx:
            wpool = kq_ctx.enter_context(tc.tile_pool(name="wpool", bufs=8))
            xload = kq_ctx.enter_context(tc.tile_pool(name="xload", bufs=2))
            lnx = kq_ctx.enter_context(tc.tile_pool(name="lnx", bufs=2))
            xtj = kq_ctx.enter_context(tc.tile_pool(name="xtj", bufs=16))
            tp_ps = kq_ctx.enter_context(
                tc.tile_pool(name="tp_ps", bufs=2, space="PSUM")
            )
            pr_ps = kq_ctx.enter_context(
                tc.tile_pool(name="pr_ps", bufs=4, space="PSUM")
            )

            def load_w(w_dram):
                w_sb = []
                for c in range(8):
                    wt = wpool.tile([P, D], FP32R, tag="w")
                    nc.sync.dma_start(wt[:], w_dram.ap()[c * P : (c + 1) * P, :])
                    w_sb.append(wt)
                return w_sb

            def project_T(x_dram, n_tiles, w_sb, out_heads, bias_sb):
                """out_heads[h][:, j] = (LN(X) @ W)^T per head (feature-major)."""
                for J in range(n_tiles // 4):
                    xt_J = [
                        xtj.tile([P, 512], FP32R, tag="xtj", name=f"xtj{c}")
                        for c in range(8)
                    ]
                    for tj in range(4):
                        t = J * 4 + tj
                        xt = xload.tile([P, D], FP32, tag="xload")
                        nc.sync.dma_start(xt[:], x_dram.ap()[t * P : (t + 1) * P, :])
                        xn = lnx.tile([P, D], FP32R, tag="lnx")
                        layernorm(lnx, xt, xn)
                        for c in range(8):
                            tp = tp_ps.tile([P, P], FP32R, tag="tp_r")
                            nc.tensor.transpose(
                                tp[:], xn[:, c * P : (c + 1) * P], ident_r[:]
                            )
                            nc.any.tensor_copy(
                                xt_J[c][:, tj * P : (tj + 1) * P], tp[:]
                            )
                    for h in range(H):
                        ps = pr_ps.tile([P, 512], FP32, tag="pr")
                        for c in range(8):
                            nc.tensor.matmul(
                                ps[:],
                                w_sb[c][:, h * HD : (h + 1) * HD],
                                xt_J[c][:],
                                start=(c == 0),
                                stop=(c == 7),
                            )
                        dst = out_heads[h][:, J * 512 : (J + 1) * 512]
                        if bias_sb is not None:
                            nc.vector.tensor_scalar_add(
                                out=dst, in0=ps[:], scalar1=bias_sb[:, h : h + 1]
                            )
                        else:
                            nc.vector.tensor_copy(dst, ps[:])

            wk_sb = load_w(Wk_d)
            project_T(Kf, 16, wk_sb, kth, bk_sb if has_qk_bias else None)
            wq_sb = load_w(Wq_d)
            project_T(Qs, 8, wq_sb, qth, bq_sb if has_qk_bias else None)

        # ============ P3: v = V @ Wv (row-major out) ============
        v_pool = act_store.enter_context(tc.tile_pool(name="vnat", bufs=16))
        vnat = [
            v_pool.tile([P, D], FP32R, tag="vnat", name=f"vnat{t}")
            for t in range(16)
        ]
        with ExitStack() as v_ctx:
            wpool2 = v_ctx.enter_context(tc.tile_pool(name="wpool2", bufs=8))
            xload2 = v_ctx.enter_context(tc.tile_pool(name="xload2", bufs=2))
            vt_pool = v_ctx.enter_context(tc.tile_pool(name="vt", bufs=8))
            tp_ps2 = v_ctx.enter_context(
                tc.tile_pool(name="tp_ps2", bufs=2, space="PSUM")
            )
            pr_ps2 = v_ctx.enter_context(
                tc.tile_pool(name="pr_ps2", bufs=4, space="PSUM")
            )
            wv_sb = []
            for c in range(8):
                wt = wpool2.tile([P, D], FP32R, tag="wv")
                nc.sync.dma_start(wt[:], Wv_d.ap()[c * P : (c + 1) * P, :])
                wv_sb.append(wt)
            for t in range(16):
                xt = xload2.tile([P, D], FP32, tag="xload2")
                nc.sync.dma_start(xt[:], Vf.ap()[t * P : (t + 1) * P, :])
                vt_blocks = []
                for c in range(8):
                    tp = tp_ps2.tile([P, P], FP32, tag="tp_f")
                    nc.tensor.transpose(
                        tp[:], xt[:, c * P : (c + 1) * P], ident_f[:]
                    )
                    vb = vt_pool.tile([P, P], FP32R, tag="vt")
                    nc.any.tensor_copy(vb[:], tp[:])
                    vt_blocks.append(vb)
                for s in range(2):
                    ps = pr_ps2.tile([P, 512], FP32, tag="pr2")
                    for c in range(8):
                        nc.tensor.matmul(
                            ps[:],
                            vt_blocks[c][:],
                            wv_sb[c][:, s * 512 : (s + 1) * 512],
                            start=(c == 0),
                            stop=(c == 7),
                        )
                    nc.vector.tensor_copy(vnat[t][:, s * 512 : (s + 1) * 512], ps[:])

        # ============ P4: attention per head (feature-major) ============
        with ExitStack() as att_ctx:
            ex_pool = att_ctx.enter_context(tc.tile_pool(name="expst", bufs=2))
            oev_pool = att_ctx.enter_context(tc.tile_pool(name="oev", bufs=2))
            srow_pool = att_ctx.enter_context(tc.tile_pool(name="srow", bufs=1))
            st_ps_pool = att_ctx.enter_context(
                tc.tile_pool(name="st_ps", bufs=2, space="PSUM")
            )
            ot_ps_pool = att_ctx.enter_context(
                tc.tile_pool(name="ot_ps", bufs=1, space="PSUM")
            )
            sm_ps_pool = att_ctx.enter_context(
                tc.tile_pool(name="sm_ps", bufs=1, space="PSUM")
            )
            for h in range(H):
                sums_ps = sm_ps_pool.tile([1, LQ], FP32, tag="sums")
                ot_ps = ot_ps_pool.tile([P, LQ], FP32, tag="otp")
                for jc in range(16):
                    st_ps = st_ps_pool.tile([P, LQ], FP32, tag="st")
                    for s in range(2):
                        nc.tensor.matmul(
                            st_ps[:, s * 512 : (s + 1) * 512],
                            kth[h][:, jc * P : (jc + 1) * P],
                            qth[h][:, s * 512 : (s + 1) * 512],
                            start=True,
                            stop=True,
                        )
                    ex = ex_pool.tile([P, LQ], FP32R, tag="ex")
                    nc.scalar.activation(ex[:], st_ps[:], AF.Exp, scale=1.0 / TEMP)
                    for s in range(2):
                        sl = slice(s * 512, (s + 1) * 512)
                        nc.tensor.matmul(
                            sums_ps[:, sl],
                            ones[:],
                            ex[:, sl],
                            start=(jc == 0),
                            stop=(jc == 15),
                        )
                        nc.tensor.matmul(
                            ot_ps[:, sl],
                            vnat[jc][:, h * HD : (h + 1) * HD],
                            ex[:, sl],
                            start=(jc == 0),
                            stop=(jc == 15),
                        )
                oev = oev_pool.tile([P, LQ], FP32R, tag="oev")
                nc.vector.tensor_copy(oev[:], ot_ps[:])
                nc.sync.dma_start(ot_dram[h], oev[:])
                srow = srow_pool.tile([1, LQ], FP32R, tag="srow")
                nc.vector.tensor_copy(srow[:], sums_ps[:])
                nc.sync.dma_start(sums_dram[h : h + 1, :], srow[:])

        act_store.close()

        # ======== D: transpose back + 1/sums, LN, Wo, gelu, residual ========
        with ExitStack() as fin_ctx:
            wo_pool = fin_ctx.enter_context(tc.tile_pool(name="wo", bufs=8))
            otl_pool = fin_ctx.enter_context(tc.tile_pool(name="otl", bufs=2))
            onat_pool = fin_ctx.enter_context(tc.tile_pool(name="onat", bufs=8))
            oln_pool = fin_ctx.enter_context(tc.tile_pool(name="oln", bufs=8))
            olnT_pool = fin_ctx.enter_context(tc.tile_pool(name="olnT", bufs=8))
            fsmall = fin_ctx.enter_context(tc.tile_pool(name="fsmall", bufs=3))
            tp2_ps = fin_ctx.enter_context(
                tc.tile_pool(name="tp2_ps", bufs=2, space="PSUM")
            )
            g_ps_pool = fin_ctx.enter_context(
                tc.tile_pool(name="g_ps", bufs=3, space="PSUM")
            )

            wo_sb = []
            for c in range(8):
                wt = wo_pool.tile([P, D], FP32R, tag="wo")
                nc.sync.dma_start(wt[:], Wo_d.ap()[c * P : (c + 1) * P, :])
                wo_sb.append(wt)

            # 1/sums, query-major: recipN[t][:, h] = 1 / sums[h, t*128:...]
            sums_all = fsmall.tile([H, LQ], FP32R, tag="sums_all")
            nc.sync.dma_start(sums_all[:], sums_dram[:])
            recipN = [
                fsmall.tile([P, H], FP32, tag=f"recipN{t}", name=f"recipN{t}")
                for t in range(8)
            ]
            for t in range(8):
                tp = tp2_ps.tile([P, H], FP32R, tag="tp_sums")
                nc.tensor.transpose(
                    tp[:], sums_all[:, t * P : (t + 1) * P], ident_r[:H, :H]
                )
                nc.vector.reciprocal(recipN[t][:], tp[:].bitcast(FP32))

            o_nat = [
                onat_pool.tile([P, D], FP32, tag="onat", name=f"onat{t}")
                for t in range(8)
            ]
            o_ln = [
                oln_pool.tile([P, D], FP32R, tag="oln", name=f"oln{t}")
                for t in range(8)
            ]
            o_lnT = [
                olnT_pool.tile([P, LQ], FP32R, tag="olnT", name=f"olnT{c}")
                for c in range(8)
            ]

            # D1: transpose O^T back to query-major, scaling by 1/sums
            for h in range(H):
                otl = otl_pool.tile([P, LQ], FP32R, tag="otl")
                nc.sync.dma_start(otl[:], ot_dram[h])
                for t in range(8):
                    tp = tp2_ps.tile([P, P], FP32R, tag="tp_d1")
                    nc.tensor.transpose(
                        tp[:], otl[:, t * P : (t + 1) * P], ident_r[:]
                    )
                    nc.vector.tensor_scalar_mul(
                        out=o_nat[t][:, h * HD : (h + 1) * HD],
                        in0=tp[:].bitcast(FP32),
                        scalar1=recipN[t][:, h : h + 1],
                    )

            # D2/D3: LayerNorm then transpose to feature-major
            if not trivial_ln:
                lng = fsmall.tile([P, D], FP32, tag="lng")
                lnb = fsmall.tile([P, D], FP32, tag="lnb")
                nc.sync.dma_start(lng[:], LNG_D.ap())
                nc.sync.dma_start(lnb[:], LNB_D.ap())
            for t in range(8):
                layernorm(fsmall, o_nat[t], o_ln[t])
                if not trivial_ln:
                    nc.vector.tensor_mul(
                        out=o_ln[t][:].bitcast(FP32),
                        in0=o_ln[t][:].bitcast(FP32),
                        in1=lng[:],
                    )
                    nc.vector.tensor_add(
                        out=o_ln[t][:],
                        in0=o_ln[t][:].bitcast(FP32),
                        in1=lnb[:],
                    )
                for c in range(8):
                    tp = tp2_ps.tile([P, P], FP32R, tag="tp_d1")
                    nc.tensor.transpose(
                        tp[:], o_ln[t][:, c * P : (c + 1) * P], ident_r[:]
                    )
                    nc.any.tensor_copy(o_lnT[c][:, t * P : (t + 1) * P], tp[:])

            # D4-7: G = O_ln @ Wo, gelu, residual, store
            for t in range(8):
                for s in range(2):
                    sl = slice(s * 512, (s + 1) * 512)
                    ps = g_ps_pool.tile([P, 512], FP32, tag="g")
                    for c in range(8):
                        nc.tensor.matmul(
                            ps[:],
                            o_lnT[c][:, t * P : (t + 1) * P],
                            wo_sb[c][:, sl],
                            start=(c == 0),
                            stop=(c == 7),
                        )
                    gel = fsmall.tile([P, 512], FP32, tag="gelu")
                    nc.scalar.activation(gel[:], ps[:], AF.Gelu)
                    outt = fsmall.tile([P, 512], FP32, tag="outsb")
                    nc.vector.tensor_add(
                        out=outt[:],
                        in0=gel[:],
                        in1=o_ln[t][:, sl].bitcast(FP32),
                    )
                    nc.sync.dma_start(OUT.ap()[t * P : (t + 1) * P, sl], outt[:])

    nc.compile()
    return nc


def _prep_host(Q, K, V, Wq, Wk, Wv, Wo, pre_g, pre_b, ln_g, ln_b):
    """Host-side preprocessing: fold pre-LN affine into weights, round fp32r."""
    pre_g = np.asarray(pre_g, np.float32)
    pre_b = np.asarray(pre_b, np.float32)
    ln_g = np.asarray(ln_g, np.float32)
    ln_b = np.asarray(ln_b, np.float32)
    Wq_eff = round_fp32r(pre_g[:, None] * np.asarray(Wq, np.float32))
    Wk_eff = round_fp32r(pre_g[:, None] * np.asarray(Wk, np.float32))
    Wv_eff = round_fp32r(np.asarray(Wv, np.float32))
    Wo_eff = round_fp32r(np.asarray(Wo, np.float32))
    has_qk_bias = bool(np.any(pre_b))
    trivial_ln = bool(np.all(ln_g == 1.0) and np.all(ln_b == 0.0))
    bq = bk = None
    if has_qk_bias:
        bq = (pre_b @ np.asarray(Wq, np.float32)).reshape(H, HD).T.copy()
        bk = (pre_b @ np.asarray(Wk, np.float32)).reshape(H, HD).T.copy()
    return Wq_eff, Wk_eff, Wv_eff, Wo_eff, has_qk_bias, trivial_ln, bq, bk, ln_g, ln_b


def kernel(Q, K, V, Wq, Wk, Wv, Wo, pre_g, pre_b, ln_g, ln_b):
    from concourse.bass_utils import run_bass_kernel_spmd

    Q = np.asarray(Q, np.float32)
    K = np.asarray(K, np.float32)
    V = np.asarray(V, np.float32)
    (Wq_e, Wk_e, Wv_e, Wo_e, has_qk_bias, trivial_ln, bq, bk, ln_g, ln_b) = _prep_host(
        Q, K, V, Wq, Wk, Wv, Wo, pre_g, pre_b, ln_g, ln_b
    )

    key = (has_qk_bias, trivial_ln)
    if key not in _PROGRAM_CACHE:
        _PROGRAM_CACHE[key] = _build_program(*key)
    nc = _PROGRAM_CACHE[key]

    ident = np.eye(P, dtype=np.float32)
    ones = np.ones((P, 1), np.float32)
    in_maps = []
    for c in range(N_CORES):
        b, half = c // 2, c % 2
        m = {
            "Qs": np.ascontiguousarray(Q[b, half * LQ : (half + 1) * LQ, :]),
            "Kf": np.ascontiguousarray(K[b]),
            "Vf": np.ascontiguousarray(V[b]),
            "Wq_r": Wq_e,
            "Wk_r": Wk_e,
            "Wv_r": Wv_e,
            "Wo_r": Wo_e,
            "ID_R": ident,
            "ID_F": ident,
            "ONES": ones,
        }
        if has_qk_bias:
            m["BQ"] = bq
            m["BK"] = bk
        if not trivial_ln:
            m["LNG_B"] = np.tile(ln_g[None, :], (P, 1))
            m["LNB_B"] = np.tile(ln_b[None, :], (P, 1))
        in_maps.append(m)

    res = run_bass_kernel_spmd(nc, in_maps, core_ids=list(range(N_CORES)))
    B = Q.shape[0]
    out = np.empty((B, 2 * LQ, D), np.float32)
    for c in range(N_CORES):
        b, half = c // 2, c % 2
        out[b, half * LQ : (half + 1) * LQ, :] = res.results[c]["OUT"]
    return out
